# revision 9
# baseline (speedup 1.0000x reference)
"""Trainium2 Bass kernel for nn_AdditiveAttention (B=32, NQ=1, NK=4096, D=512, H=256).

Data-parallel over 8 NeuronCores: each core owns 4 batches. Per core:
  kprojT[h, t] = sum_d W_k[d, h] * keys[b, t, d]      (PE, bf16, W_k stationary)
  featT        = tanh(kprojT + qbias_b)               (ACT, bias fused, bf16 out)
  scores       = w_v . featT                          (PE col-tiled matvec: the 4
                                                       batches' scores land on
                                                       rows 0/32/64/96 of ONE
                                                       PSUM tile, concurrently)
  out[b, t]    = softmax_t(scores) * values[b, t]     (exp straight from PSUM with
                                                       accum_out denominators;
                                                       scores are O(4) so no
                                                       max-subtract)

Key points vs the naive layout:
  * qbias (queries @ W_q) is computed on HOST (tiny) - no f32 qproj on device.
  * All softmax-side ops (exp, *values, *1/denom) run on [128, 1024] tiles
    with the 4 batches stacked on partitions 32b - ACT/DVE cost is driven by
    the free-dim size, so processing 4 rows together is 4x cheaper than
    per-batch [1, tok] row ops.
  * The matvec uses tile_position=(0, 32b) col-tiling so the 4 batches'
    matvec matmuls execute concurrently in disjoint 32-col groups of the
    PE array (~4x faster than sequential full-width matmuls).
  * The matvec for chunk c is emitted AFTER kproj of chunk c+1's first batch
    so the last tanh's latency hides under kproj matmuls.
  * Keys arrive as 32 x 0.5MB DMAs so the first kproj can start ~1.5us after
    the first DMA issues; a few bf16 warmup matmuls on memset data bridge the
    preamble and keep the PE HAM clock-gate warming.
"""

import numpy as np
import ml_dtypes

N_CORES = 8
B, NQ, NK, D, H = 32, 1, 4096, 512, 256
B_LOC = B // N_CORES  # 4 batches per core
KT = D // 128         # 4 contraction tiles
HT = H // 128         # 2 hidden tiles
CH = 1024             # token chunk (2 PSUM banks of f32)
NCH = NK // CH        # 4 chunks
N_WARM = 16           # HAM warmup matmuls (bridge until keys arrive)


def _install_profile_hook():
    """Make trace=True / BASS_TRACE=1 usable when the image's antenv lacks
    axon_hooks (degrades silently if anything is missing)."""
    try:
        from antenv import axon_hooks  # noqa: F401
        return
    except ImportError:
        pass
    try:
        import sys
        import types

        import antenv
        from trn_agent_boot.trn_boot import _ntff_profile_via_ctypes

        mod = types.ModuleType("antenv.axon_hooks")
        mod._h = None
        mod.set_axon_ntff_profile_hook = lambda h: setattr(mod, "_h", h)
        mod.get_axon_ntff_profile_hook = lambda: mod._h
        antenv.axon_hooks = mod
        sys.modules["antenv.axon_hooks"] = mod
        mod._h = _ntff_profile_via_ctypes("/opt/axon/libaxon_pjrt.so")
    except Exception:
        pass


def build_nc():
    import concourse.tile as tile
    from concourse import bacc, mybir

    f32 = mybir.dt.float32
    f16 = mybir.dt.bfloat16
    Act = mybir.ActivationFunctionType
    AX = mybir.AxisListType.X

    nc = bacc.Bacc("TRN2", target_bir_lowering=False, debug=False,
                   num_devices=N_CORES)

    keysT_ext = nc.dram_tensor("keysT", [B_LOC, D, NK], f16, kind="ExternalInput")
    qbias_ext = nc.dram_tensor("qbias", [128, HT * B_LOC], f32, kind="ExternalInput")
    vals_ext = nc.dram_tensor("vals", [B_LOC, NK], f32, kind="ExternalInput")
    wk_ext = nc.dram_tensor("wk", [128, KT * H], f16, kind="ExternalInput")
    wv_ext = nc.dram_tensor("wv", [128, B_LOC * HT * 32], f16, kind="ExternalInput")
    out_ext = nc.dram_tensor("out", [B_LOC, NK], f32, kind="ExternalOutput")

    # [B_LOC, D, NK] viewed so one DMA can pull [128 part, KT, ntok]
    keys3d = keysT_ext.ap().rearrange("b (k p) n -> b k p n", p=128)

    with tile.TileContext(nc) as tc:
        with (
            tc.tile_pool(name="keys", bufs=16) as keys_pool,
            tc.tile_pool(name="feat", bufs=12) as feat_pool,
            tc.tile_pool(name="static", bufs=1) as st,
            tc.tile_pool(name="kp", bufs=2, space="PSUM") as kp_pool,
            tc.tile_pool(name="sc", bufs=2, space="PSUM") as sc_pool,
        ):
            # ---- loads first: keys chunks on the sync HWDGE queue (16
            # engines); small weights ride the scalar queue in parallel
            # (before the dummy activations so the ACT table load doesn't
            # delay them); vals on the gpsimd queue ----
            kt_tiles = {}
            for c in range(NCH):
                for b in range(B_LOC):
                    for j in range(2):
                        t = keys_pool.tile([128, KT, 512], f16, tag="kt")
                        s0 = c * CH + j * 512
                        nc.sync.dma_start(
                            t[:], keys3d[b, :, :, s0:s0 + 512]
                            .rearrange("k p n -> p k n"))
                        kt_tiles[(b, c, j)] = t
            wk_sb = st.tile([128, KT, H], f16, tag="wk")
            nc.scalar.dma_start(wk_sb[:], wk_ext.ap())
            qbias_sb = st.tile([128, HT, B_LOC], f32, tag="qbias")
            nc.scalar.dma_start(qbias_sb[:], qbias_ext.ap())
            # w_v per (b, h) as a [128, 32] stationary with the vector in
            # group-col 0, so batch b's scores land on PSUM partition 32*b
            wv_sb = st.tile([128, B_LOC, HT, 32], f16, tag="wv")
            nc.scalar.dma_start(wv_sb[:], wv_ext.ap())
            vals_sb = st.tile([128, NK], f32, tag="vals")
            nc.gpsimd.memset(vals_sb[:], 0.0)
            nc.gpsimd.dma_start(
                vals_sb.rearrange("(b p) n -> b p n", p=32)[:, 0, :],
                vals_ext.ap())

            # ---- HAM warmup on memset data: PE activity needs no DMA, so
            # the clock-gate starts warming before the first real matmul ----
            wtile = st.tile([128, 256], f16, tag="warm_in")
            nc.vector.memset(wtile[:], 1.0)
            warm_ps = kp_pool.tile([128, CH], f32, tag="kp")
            for w in range(N_WARM):
                nc.tensor.matmul(warm_ps[:, 0:256], wtile[:, 0:128], wtile[:],
                                 start=(w == 0), stop=(w == N_WARM - 1))
            warm_out = st.tile([128, 1], f32, tag="warm")
            nc.vector.reduce_max(warm_out[:], warm_ps[:, 0:256], axis=AX)
            # dummy tanh/exp: force the ACT table load (~2.7us) to happen
            # during the ramp instead of before the first real tanh
            dummy_sb = st.tile([128, 1], f32, tag="dummy")
            nc.scalar.activation(dummy_sb[:], wtile[:, 0:1], Act.Tanh)
            nc.scalar.activation(dummy_sb[:], wtile[:, 0:1], Act.Exp)

            # ---- per-core softmax state (batch b on partition 32*b) ----
            esc_sb = st.tile([128, NK], f32, tag="esc")       # exp(scores)*vals
            psum_sb = st.tile([128, 2 * NCH], f32, tag="psums")  # half-chunk denoms
            ssum_sb = st.tile([128, 1], f32, tag="ssum")
            recip_sb = st.tile([128, 1], f32, tag="recip")

            fts = {}      # (b, h, c) -> feat tile
            sc_tiles = {}  # c -> scores PSUM tile

            def emit_kproj(c, b):
                last = (c == NCH - 1 and b == B_LOC - 1)
                for h in range(HT):
                    ps = kp_pool.tile([128, CH], f32, tag="kp")
                    for j in range(2):
                        src = kt_tiles[(b, c, j)]
                        for k in range(KT):
                            nc.tensor.matmul(
                                ps[:, j * 512:(j + 1) * 512],
                                wk_sb[:, k, h * 128:(h + 1) * 128],
                                src[:, k, :],
                                start=(k == 0), stop=(k == KT - 1),
                            )
                    ft = feat_pool.tile([128, CH], f16, tag="ft")
                    if last:
                        # split the final tanh so the last matvec + exp can
                        # start on the first half ~0.7us earlier
                        for j in range(2):
                            nc.scalar.activation(
                                ft[:, j * 512:(j + 1) * 512],
                                ps[:, j * 512:(j + 1) * 512], Act.Tanh,
                                bias=qbias_sb[:, h, b:b + 1])
                    else:
                        nc.scalar.activation(ft[:], ps[:], Act.Tanh,
                                             bias=qbias_sb[:, h, b:b + 1])
                    fts[(b, h, c)] = ft

            def emit_matvec(c):
                # col-tiled: the 4 batches' matmuls target disjoint 32-col
                # groups of the PE array and run concurrently
                sc = sc_tiles[c]
                for h in range(HT):
                    for j in range(2):
                        for b in range(B_LOC):
                            nc.tensor.matmul(
                                sc[32 * b:32 * b + 32, j * 512:(j + 1) * 512],
                                wv_sb[:, b, h, :],
                                fts[(b, h, c)][:, j * 512:(j + 1) * 512],
                                start=(h == 0), stop=(h == HT - 1),
                                tile_position=(0, 32 * b),
                                skip_group_check=True,
                            )
                # exp/mul per 512-half: the j0 half unblocks while the j1
                # matvec is still streaming, shortening the serial tail
                for j in range(2):
                    cs = c * CH + j * 512
                    nc.scalar.activation(esc_sb[:, cs:cs + 512],
                                         sc[:, j * 512:(j + 1) * 512], Act.Exp,
                                         accum_out=psum_sb[:, 2 * c + j:2 * c + j + 1])
                    nc.vector.tensor_mul(esc_sb[:, cs:cs + 512],
                                         esc_sb[:, cs:cs + 512],
                                         vals_sb[:, cs:cs + 512])

            for c in range(NCH):
                sc_tiles[c] = sc_pool.tile([128, CH], f32, tag="sc", name="sc")
                for b in range(B_LOC):
                    emit_kproj(c, b)
                    # defer chunk c-1's matvec until after kproj(c, b0) so
                    # the last tanh's latency hides under kproj matmuls
                    if b == 0 and c > 0:
                        emit_matvec(c - 1)
            emit_matvec(NCH - 1)

            # softmax denominator; scale split across DVE/GpSimd/ACT
            nc.vector.reduce_sum(ssum_sb[:], psum_sb[:], axis=AX)
            nc.vector.reciprocal(recip_sb[:], ssum_sb[:])
            scale_engines = [nc.vector, nc.gpsimd, nc.vector, nc.scalar]
            for g in range(NCH):
                gs = g * CH
                if scale_engines[g] is nc.scalar:
                    nc.scalar.mul(esc_sb[:, gs:gs + CH], esc_sb[:, gs:gs + CH],
                                  recip_sb[:])
                else:
                    scale_engines[g].tensor_scalar_mul(
                        esc_sb[:, gs:gs + CH], esc_sb[:, gs:gs + CH],
                        recip_sb[:])
            # one partition-strided DMA stores all 4 output rows
            nc.sync.dma_start(
                out_ext.ap(),
                esc_sb.rearrange("(b p) n -> b p n", p=32)[:, 0, :])

    nc.compile()
    return nc


def shard_inputs(queries, keys, values, W_q, W_k, w_v):
    queries = np.asarray(queries, np.float32)
    keys = np.asarray(keys, np.float32)
    values = np.asarray(values, np.float32)
    W_q = np.asarray(W_q, np.float64)
    W_k = np.asarray(W_k, np.float32)
    w_v = np.asarray(w_v, np.float32)

    def merge_kt(w, ncol):  # [KT*128, ncol] -> [128, KT*ncol] partition-major
        return np.ascontiguousarray(
            w.reshape(KT, 128, ncol).transpose(1, 0, 2).reshape(128, KT * ncol))

    wk2 = merge_kt(W_k, H).astype(ml_dtypes.bfloat16)
    wv2 = np.zeros((128, B_LOC, HT, 32), np.float32)
    for b in range(B_LOC):
        for h in range(HT):
            wv2[:, b, h, 0] = w_v[h * 128:(h + 1) * 128]
    wv2 = wv2.reshape(128, B_LOC * HT * 32).astype(ml_dtypes.bfloat16)

    # qbias on host (tiny): [B, H] = queries @ W_q, exact in f64
    qb_all = (queries[:, 0, :].astype(np.float64) @ W_q).astype(np.float32)

    in_maps = []
    for i in range(N_CORES):
        b0, b1 = i * B_LOC, (i + 1) * B_LOC
        qb = np.zeros((128, HT, B_LOC), np.float32)
        for b in range(B_LOC):
            for h in range(HT):
                qb[:, h, b] = qb_all[b0 + b, h * 128:(h + 1) * 128]
        in_maps.append({
            "keysT": np.ascontiguousarray(
                keys[b0:b1].transpose(0, 2, 1)).astype(ml_dtypes.bfloat16),
            "qbias": qb.reshape(128, HT * B_LOC),
            "vals": np.ascontiguousarray(values[b0:b1, :, 0]),
            "wk": wk2, "wv": wv2,
        })
    return in_maps


_NC_CACHE = {}


def run(in_maps, trace=False, tmpdir=None):
    from concourse.bass_utils import run_bass_kernel_spmd

    _install_profile_hook()
    try:
        # no artifact bucket inside the container; keep traces local
        import concourse.bass_utils as bu
        bu.upload_artifacts = lambda d: "local://" + d
    except Exception:
        pass
    if "nc" not in _NC_CACHE:
        _NC_CACHE["nc"] = build_nc()
    nc = _NC_CACHE["nc"]
    return run_bass_kernel_spmd(nc, in_maps, core_ids=list(range(N_CORES)),
                                trace=trace, tmpdir=tmpdir)


def kernel(queries, keys, values, W_q, W_k, w_v):
    in_maps = shard_inputs(queries, keys, values, W_q, W_k, w_v)
    res = run(in_maps)
    return np.concatenate([res.results[i]["out"] for i in range(N_CORES)], axis=0)


# revision 11
# speedup vs baseline: 1.1740x; 1.1740x over previous
"""Trainium2 Bass kernel for nn_AdditiveAttention (B=32, NQ=1, NK=4096, D=512, H=256).

Data-parallel over 8 NeuronCores: each core owns 4 batches. Per core:
  kprojT[h, t] = sum_d W_k[d, h] * keys[b, t, d]      (PE, bf16, W_k stationary)
  featT        = tanh(kprojT + qbias_b)               (ACT, bias fused, bf16 out)
  scores       = w_v . featT                          (PE col-tiled matvec: the 4
                                                       batches' scores land on
                                                       rows 0/32/64/96 of ONE
                                                       PSUM tile, concurrently)
  out[b, t]    = softmax_t(scores) * values[b, t]     (exp straight from PSUM with
                                                       accum_out denominators;
                                                       scores are O(4) so no
                                                       max-subtract)

Key points vs the naive layout:
  * qbias (queries @ W_q) is computed on HOST (tiny) - no f32 qproj on device.
  * All softmax-side ops (exp, *values, *1/denom) run on [128, 1024] tiles
    with the 4 batches stacked on partitions 32b - ACT/DVE cost is driven by
    the free-dim size, so processing 4 rows together is 4x cheaper than
    per-batch [1, tok] row ops.
  * The matvec uses tile_position=(0, 32b) col-tiling so the 4 batches'
    matvec matmuls execute concurrently in disjoint 32-col groups of the
    PE array (~4x faster than sequential full-width matmuls).
  * The matvec for chunk c is emitted AFTER kproj of chunk c+1's first batch
    so the last tanh's latency hides under kproj matmuls.
  * Keys arrive as 32 x 0.5MB DMAs so the first kproj can start ~1.5us after
    the first DMA issues; a few bf16 warmup matmuls on memset data bridge the
    preamble and keep the PE HAM clock-gate warming.
"""

import numpy as np
import ml_dtypes

N_CORES = 8
B, NQ, NK, D, H = 32, 1, 4096, 512, 256
B_LOC = B // N_CORES  # 4 batches per core
KT = D // 128         # 4 contraction tiles
HT = H // 128         # 2 hidden tiles
CH = 1024             # token chunk (2 PSUM banks of f32)
NCH = NK // CH        # 4 chunks
N_WARM = 20           # HAM warmup matmuls (bridge until keys arrive)


def _install_profile_hook():
    """Make trace=True / BASS_TRACE=1 usable when the image's antenv lacks
    axon_hooks (degrades silently if anything is missing)."""
    try:
        from antenv import axon_hooks  # noqa: F401
        return
    except ImportError:
        pass
    try:
        import sys
        import types

        import antenv
        from trn_agent_boot.trn_boot import _ntff_profile_via_ctypes

        mod = types.ModuleType("antenv.axon_hooks")
        mod._h = None
        mod.set_axon_ntff_profile_hook = lambda h: setattr(mod, "_h", h)
        mod.get_axon_ntff_profile_hook = lambda: mod._h
        antenv.axon_hooks = mod
        sys.modules["antenv.axon_hooks"] = mod
        mod._h = _ntff_profile_via_ctypes("/opt/axon/libaxon_pjrt.so")
    except Exception:
        pass


def build_nc():
    import concourse.tile as tile
    from concourse import bacc, mybir

    f32 = mybir.dt.float32
    f16 = mybir.dt.bfloat16
    Act = mybir.ActivationFunctionType
    AX = mybir.AxisListType.X

    nc = bacc.Bacc("TRN2", target_bir_lowering=False, debug=False,
                   num_devices=N_CORES)

    keysT_ext = nc.dram_tensor("keysT", [B_LOC, D, NK], f16, kind="ExternalInput")
    qbias_ext = nc.dram_tensor("qbias", [128, HT * B_LOC], f32, kind="ExternalInput")
    vals_ext = nc.dram_tensor("vals", [B_LOC, NK], f32, kind="ExternalInput")
    wk_ext = nc.dram_tensor("wk", [128, KT * H], f16, kind="ExternalInput")
    wv_ext = nc.dram_tensor("wv", [128, B_LOC * HT * 32], f16, kind="ExternalInput")
    out_ext = nc.dram_tensor("out", [B_LOC, NK], f32, kind="ExternalOutput")

    # [B_LOC, D, NK] viewed so one DMA can pull [128 part, KT, ntok]
    keys3d = keysT_ext.ap().rearrange("b (k p) n -> b k p n", p=128)

    with tile.TileContext(nc) as tc:
        with (
            tc.tile_pool(name="keys", bufs=16) as keys_pool,
            tc.tile_pool(name="feat", bufs=12) as feat_pool,
            tc.tile_pool(name="static", bufs=1) as st,
            tc.tile_pool(name="kp", bufs=2, space="PSUM") as kp_pool,
            tc.tile_pool(name="sc", bufs=2, space="PSUM") as sc_pool,
        ):
            # ---- loads first: keys chunks on the sync HWDGE queue (16
            # engines); small weights ride the scalar queue in parallel
            # (before the dummy activations so the ACT table load doesn't
            # delay them); vals on the gpsimd queue ----
            kt_tiles = {}
            for c in range(NCH):
                for b in range(B_LOC):
                    for j in range(2):
                        t = keys_pool.tile([128, KT, 512], f16, tag="kt")
                        s0 = c * CH + j * 512
                        nc.sync.dma_start(
                            t[:], keys3d[b, :, :, s0:s0 + 512]
                            .rearrange("k p n -> p k n"))
                        kt_tiles[(b, c, j)] = t
            wk_sb = st.tile([128, KT, H], f16, tag="wk")
            nc.scalar.dma_start(wk_sb[:], wk_ext.ap())
            qbias_sb = st.tile([128, HT, B_LOC], f32, tag="qbias")
            nc.scalar.dma_start(qbias_sb[:], qbias_ext.ap())
            # w_v per (b, h) as a [128, 32] stationary with the vector in
            # group-col 0, so batch b's scores land on PSUM partition 32*b
            wv_sb = st.tile([128, B_LOC, HT, 32], f16, tag="wv")
            nc.scalar.dma_start(wv_sb[:], wv_ext.ap())
            vals_sb = st.tile([128, NK], f32, tag="vals")
            nc.gpsimd.memset(vals_sb[:], 0.0)
            nc.gpsimd.dma_start(
                vals_sb.rearrange("(b p) n -> b p n", p=32)[:, 0, :],
                vals_ext.ap())

            # ---- HAM warmup on memset data: PE activity needs no DMA, so
            # the clock-gate starts warming before the first real matmul ----
            wtile = st.tile([128, 256], f16, tag="warm_in")
            nc.vector.memset(wtile[:], 1.0)
            warm_ps = kp_pool.tile([128, CH], f32, tag="kp")
            for w in range(N_WARM):
                nc.tensor.matmul(warm_ps[:, 0:256], wtile[:, 0:128], wtile[:],
                                 start=(w == 0), stop=(w == N_WARM - 1))
            warm_out = st.tile([128, 1], f32, tag="warm")
            nc.vector.reduce_max(warm_out[:], warm_ps[:, 0:256], axis=AX)
            # dummy tanh/exp: force the ACT table load (~2.7us) to happen
            # during the ramp instead of before the first real tanh
            dummy_sb = st.tile([128, 1], f32, tag="dummy")
            nc.scalar.activation(dummy_sb[:], wtile[:, 0:1], Act.Tanh)
            nc.scalar.activation(dummy_sb[:], wtile[:, 0:1], Act.Exp)

            # ---- per-core softmax state (batch b on partition 32*b) ----
            esc_sb = st.tile([128, NK], f32, tag="esc")       # exp(scores)*vals
            psum_sb = st.tile([128, 2 * NCH], f32, tag="psums")  # half-chunk denoms
            ssum_sb = st.tile([128, 1], f32, tag="ssum")
            recip_sb = st.tile([128, 1], f32, tag="recip")

            fts = {}      # (b, h, c) -> feat tile
            sc_tiles = {}  # c -> scores PSUM tile

            def emit_kproj(c, b):
                last = (c == NCH - 1 and b == B_LOC - 1)
                for h in range(HT):
                    ps = kp_pool.tile([128, CH], f32, tag="kp")
                    for j in range(2):
                        src = kt_tiles[(b, c, j)]
                        for k in range(KT):
                            nc.tensor.matmul(
                                ps[:, j * 512:(j + 1) * 512],
                                wk_sb[:, k, h * 128:(h + 1) * 128],
                                src[:, k, :],
                                start=(k == 0), stop=(k == KT - 1),
                            )
                    ft = feat_pool.tile([128, CH], f16, tag="ft")
                    if last:
                        # split the final tanh so the last matvec + exp can
                        # start on the first half ~0.7us earlier
                        for j in range(2):
                            nc.scalar.activation(
                                ft[:, j * 512:(j + 1) * 512],
                                ps[:, j * 512:(j + 1) * 512], Act.Tanh,
                                bias=qbias_sb[:, h, b:b + 1])
                    else:
                        nc.scalar.activation(ft[:], ps[:], Act.Tanh,
                                             bias=qbias_sb[:, h, b:b + 1])
                    fts[(b, h, c)] = ft

            def emit_matvec(c):
                # col-tiled: the 4 batches' matmuls target disjoint 32-col
                # groups of the PE array and run concurrently
                sc = sc_tiles[c]
                for h in range(HT):
                    for j in range(2):
                        for b in range(B_LOC):
                            nc.tensor.matmul(
                                sc[32 * b:32 * b + 32, j * 512:(j + 1) * 512],
                                wv_sb[:, b, h, :],
                                fts[(b, h, c)][:, j * 512:(j + 1) * 512],
                                start=(h == 0), stop=(h == HT - 1),
                                tile_position=(0, 32 * b),
                                skip_group_check=True,
                            )
                # exp/mul per 512-half: the j0 half unblocks while the j1
                # matvec is still streaming, shortening the serial tail
                for j in range(2):
                    cs = c * CH + j * 512
                    nc.scalar.activation(esc_sb[:, cs:cs + 512],
                                         sc[:, j * 512:(j + 1) * 512], Act.Exp,
                                         accum_out=psum_sb[:, 2 * c + j:2 * c + j + 1])
                    nc.vector.tensor_mul(esc_sb[:, cs:cs + 512],
                                         esc_sb[:, cs:cs + 512],
                                         vals_sb[:, cs:cs + 512])

            for c in range(NCH):
                sc_tiles[c] = sc_pool.tile([128, CH], f32, tag="sc", name="sc")
                for b in range(B_LOC):
                    emit_kproj(c, b)
                    # defer chunk c-1's matvec until after kproj(c, b0) so
                    # the last tanh's latency hides under kproj matmuls
                    if b == 0 and c > 0:
                        emit_matvec(c - 1)
            emit_matvec(NCH - 1)

            # softmax denominator; scale split across DVE/GpSimd/ACT
            nc.vector.reduce_sum(ssum_sb[:], psum_sb[:], axis=AX)
            nc.vector.reciprocal(recip_sb[:], ssum_sb[:])
            # (GpSimd tensor ops are ucode-slow ~15ns/col - never use here)
            for g in range(NCH):
                gs = g * CH
                if g % 2 == 1:
                    nc.scalar.mul(esc_sb[:, gs:gs + CH], esc_sb[:, gs:gs + CH],
                                  recip_sb[:])
                else:
                    nc.vector.tensor_scalar_mul(
                        esc_sb[:, gs:gs + CH], esc_sb[:, gs:gs + CH],
                        recip_sb[:])
            # one partition-strided DMA stores all 4 output rows
            nc.sync.dma_start(
                out_ext.ap(),
                esc_sb.rearrange("(b p) n -> b p n", p=32)[:, 0, :])

    nc.compile()
    return nc


def shard_inputs(queries, keys, values, W_q, W_k, w_v):
    queries = np.asarray(queries, np.float32)
    keys = np.asarray(keys, np.float32)
    values = np.asarray(values, np.float32)
    W_q = np.asarray(W_q, np.float64)
    W_k = np.asarray(W_k, np.float32)
    w_v = np.asarray(w_v, np.float32)

    def merge_kt(w, ncol):  # [KT*128, ncol] -> [128, KT*ncol] partition-major
        return np.ascontiguousarray(
            w.reshape(KT, 128, ncol).transpose(1, 0, 2).reshape(128, KT * ncol))

    wk2 = merge_kt(W_k, H).astype(ml_dtypes.bfloat16)
    wv2 = np.zeros((128, B_LOC, HT, 32), np.float32)
    for b in range(B_LOC):
        for h in range(HT):
            wv2[:, b, h, 0] = w_v[h * 128:(h + 1) * 128]
    wv2 = wv2.reshape(128, B_LOC * HT * 32).astype(ml_dtypes.bfloat16)

    # qbias on host (tiny): [B, H] = queries @ W_q, exact in f64
    qb_all = (queries[:, 0, :].astype(np.float64) @ W_q).astype(np.float32)

    in_maps = []
    for i in range(N_CORES):
        b0, b1 = i * B_LOC, (i + 1) * B_LOC
        qb = np.zeros((128, HT, B_LOC), np.float32)
        for b in range(B_LOC):
            for h in range(HT):
                qb[:, h, b] = qb_all[b0 + b, h * 128:(h + 1) * 128]
        in_maps.append({
            "keysT": np.ascontiguousarray(
                keys[b0:b1].transpose(0, 2, 1)).astype(ml_dtypes.bfloat16),
            "qbias": qb.reshape(128, HT * B_LOC),
            "vals": np.ascontiguousarray(values[b0:b1, :, 0]),
            "wk": wk2, "wv": wv2,
        })
    return in_maps


_NC_CACHE = {}


def run(in_maps, trace=False, tmpdir=None):
    from concourse.bass_utils import run_bass_kernel_spmd

    _install_profile_hook()
    try:
        # no artifact bucket inside the container; keep traces local
        import concourse.bass_utils as bu
        bu.upload_artifacts = lambda d: "local://" + d
    except Exception:
        pass
    if "nc" not in _NC_CACHE:
        _NC_CACHE["nc"] = build_nc()
    nc = _NC_CACHE["nc"]
    return run_bass_kernel_spmd(nc, in_maps, core_ids=list(range(N_CORES)),
                                trace=trace, tmpdir=tmpdir)


def kernel(queries, keys, values, W_q, W_k, w_v):
    in_maps = shard_inputs(queries, keys, values, W_q, W_k, w_v)
    res = run(in_maps)
    return np.concatenate([res.results[i]["out"] for i in range(N_CORES)], axis=0)


# revision 17
# speedup vs baseline: 1.3139x; 1.1192x over previous
"""Trainium2 Bass kernel for nn_AdditiveAttention (B=32, NQ=1, NK=4096, D=512, H=256).

Data-parallel over 8 NeuronCores: each core owns 4 batches. Per core:
  kprojT[h, t] = sum_d W_k[d, h] * keys[b, t, d]      (PE, bf16, W_k stationary)
  featT        = tanh(kprojT + qbias_b)               (ACT, bias fused, bf16 out)
  scores       = w_v . featT                          (PE col-tiled matvec: the 4
                                                       batches' scores land on
                                                       rows 0/32/64/96 of ONE
                                                       PSUM tile, concurrently)
  out[b, t]    = softmax_t(scores) * values[b, t]     (exp straight from PSUM with
                                                       accum_out denominators;
                                                       scores are O(4) so no
                                                       max-subtract)

Key points vs the naive layout:
  * qbias (queries @ W_q) is computed on HOST (tiny) - no f32 qproj on device.
  * All softmax-side ops (exp, *values, *1/denom) run on [128, 1024] tiles
    with the 4 batches stacked on partitions 32b - ACT/DVE cost is driven by
    the free-dim size, so processing 4 rows together is 4x cheaper than
    per-batch [1, tok] row ops.
  * The matvec uses tile_position=(0, 32b) col-tiling so the 4 batches'
    matvec matmuls execute concurrently in disjoint 32-col groups of the
    PE array (~4x faster than sequential full-width matmuls).
  * The matvec for chunk c is emitted AFTER kproj of chunk c+1's first batch
    so the last tanh's latency hides under kproj matmuls.
  * Keys arrive as 32 x 0.5MB DMAs so the first kproj can start ~1.5us after
    the first DMA issues; a few bf16 warmup matmuls on memset data bridge the
    preamble and keep the PE HAM clock-gate warming.
"""

import numpy as np
import ml_dtypes

N_CORES = 8
B, NQ, NK, D, H = 32, 1, 4096, 512, 256
B_LOC = B // N_CORES  # 4 batches per core
KT = D // 128         # 4 contraction tiles
HT = H // 128         # 2 hidden tiles
CH = 1024             # token chunk (2 PSUM banks of f32)
NCH = NK // CH        # 4 chunks
N_WARM = 20           # HAM warmup matmuls (bridge until keys arrive)


def _install_profile_hook():
    """Make trace=True / BASS_TRACE=1 usable when the image's antenv lacks
    axon_hooks (degrades silently if anything is missing)."""
    try:
        from antenv import axon_hooks  # noqa: F401
        return
    except ImportError:
        pass
    try:
        import sys
        import types

        import antenv
        from trn_agent_boot.trn_boot import _ntff_profile_via_ctypes

        mod = types.ModuleType("antenv.axon_hooks")
        mod._h = None
        mod.set_axon_ntff_profile_hook = lambda h: setattr(mod, "_h", h)
        mod.get_axon_ntff_profile_hook = lambda: mod._h
        antenv.axon_hooks = mod
        sys.modules["antenv.axon_hooks"] = mod
        mod._h = _ntff_profile_via_ctypes("/opt/axon/libaxon_pjrt.so")
    except Exception:
        pass


def build_nc():
    import concourse.tile as tile
    from concourse import bacc, mybir

    f32 = mybir.dt.float32
    f16 = mybir.dt.bfloat16
    Act = mybir.ActivationFunctionType
    AX = mybir.AxisListType.X

    nc = bacc.Bacc("TRN2", target_bir_lowering=False, debug=False,
                   num_devices=N_CORES)

    f8 = mybir.dt.float8e4
    # keys split: d in [0,256) as e4m3 (one DoubleRow matmul, 2x rate),
    # d in [256,512) as bf16 (two regular matmuls). rel err ~1.6e-2 < 2e-2.
    keys8_ext = nc.dram_tensor("keys8", [B_LOC, 2, 128, NK], f8,
                               kind="ExternalInput")
    keys16_ext = nc.dram_tensor("keys16", [B_LOC, 2, 128, NK], f16,
                                kind="ExternalInput")
    qbias_ext = nc.dram_tensor("qbias", [128, HT * B_LOC], f32, kind="ExternalInput")
    vals_ext = nc.dram_tensor("vals", [B_LOC, NK], f32, kind="ExternalInput")
    wk8_ext = nc.dram_tensor("wk8", [128, 2 * H], f8, kind="ExternalInput")
    wk16_ext = nc.dram_tensor("wk16", [128, 2 * H], f16, kind="ExternalInput")
    wv_ext = nc.dram_tensor("wv", [128, B_LOC * HT * 32], f16, kind="ExternalInput")
    out_ext = nc.dram_tensor("out", [B_LOC, NK], f32, kind="ExternalOutput")

    keys8_4d = keys8_ext.ap()    # [B_LOC, slot, 128, NK]
    keys16_4d = keys16_ext.ap()  # [B_LOC, k2, 128, NK]

    with tile.TileContext(nc) as tc:
        with (
            tc.tile_pool(name="keys", bufs=16) as keys_pool,
            tc.tile_pool(name="feat", bufs=12) as feat_pool,
            tc.tile_pool(name="static", bufs=1) as st,
            tc.tile_pool(name="kp", bufs=2, space="PSUM") as kp_pool,
            tc.tile_pool(name="sc", bufs=2, space="PSUM") as sc_pool,
        ):
            # ---- loads first: keys chunks on the sync HWDGE queue (16
            # engines); small weights ride the scalar queue in parallel
            # (before the dummy activations so the ACT table load doesn't
            # delay them); vals on the gpsimd queue ----
            kt8_tiles = {}
            kt16_tiles = {}
            for c in range(NCH):
                for b in range(B_LOC):
                    for j in range(2):
                        s0 = c * CH + j * 512
                        t8 = keys_pool.tile([128, 2, 512], f8, tag="kt8",
                                            name="kt8")
                        nc.sync.dma_start(
                            t8[:], keys8_4d[b, :, :, s0:s0 + 512]
                            .rearrange("s p n -> p s n"))
                        kt8_tiles[(b, c, j)] = t8
                        t16 = keys_pool.tile([128, 2, 512], f16, tag="kt16",
                                             name="kt16")
                        nc.sync.dma_start(
                            t16[:], keys16_4d[b, :, :, s0:s0 + 512]
                            .rearrange("k p n -> p k n"))
                        kt16_tiles[(b, c, j)] = t16
            wk8_sb = st.tile([128, 2, H], f8, tag="wk8")
            nc.scalar.dma_start(wk8_sb[:], wk8_ext.ap())
            wk16_sb = st.tile([128, 2, H], f16, tag="wk16")
            nc.scalar.dma_start(wk16_sb[:], wk16_ext.ap())
            qbias_sb = st.tile([128, HT, B_LOC], f32, tag="qbias")
            nc.scalar.dma_start(qbias_sb[:], qbias_ext.ap())
            # w_v per (b, h) as a [128, 32] stationary with the vector in
            # group-col 0, so batch b's scores land on PSUM partition 32*b
            wv_sb = st.tile([128, B_LOC, HT, 32], f16, tag="wv")
            nc.scalar.dma_start(wv_sb[:], wv_ext.ap())
            vals_sb = st.tile([128, NK], f32, tag="vals")
            nc.gpsimd.memset(vals_sb[:], 0.0)
            nc.gpsimd.dma_start(
                vals_sb.rearrange("(b p) n -> b p n", p=32)[:, 0, :],
                vals_ext.ap())

            # ---- HAM warmup on memset data: PE activity needs no DMA, so
            # the clock-gate starts warming before the first real matmul ----
            wtile = st.tile([128, 256], f16, tag="warm_in")
            nc.vector.memset(wtile[:], 1.0)
            warm_ps = kp_pool.tile([128, CH], f32, tag="kp")
            for w in range(N_WARM):
                nc.tensor.matmul(warm_ps[:, 0:256], wtile[:, 0:128], wtile[:],
                                 start=(w == 0), stop=(w == N_WARM - 1))
            warm_out = st.tile([128, 1], f32, tag="warm")
            nc.vector.reduce_max(warm_out[:], warm_ps[:, 0:256], axis=AX)
            # dummy tanh/exp: force the ACT table load (~2.7us) to happen
            # during the ramp instead of before the first real tanh
            dummy_sb = st.tile([128, 1], f32, tag="dummy")
            nc.scalar.activation(dummy_sb[:], wtile[:, 0:1], Act.Tanh)
            nc.scalar.activation(dummy_sb[:], wtile[:, 0:1], Act.Exp)

            # ---- per-core softmax state (batch b on partition 32*b) ----
            esc_sb = st.tile([128, NK], f32, tag="esc")       # exp(scores)*vals
            psum_sb = st.tile([128, 2 * NCH], f32, tag="psums")  # half-chunk denoms
            ssum_sb = st.tile([128, 1], f32, tag="ssum")
            recip_sb = st.tile([128, 1], f32, tag="recip")

            fts = {}      # (b, h, c) -> feat tile
            sc_tiles = {}  # c -> scores PSUM tile

            def emit_kproj(c, b):
                last = (c == NCH - 1 and b == B_LOC - 1)
                for h in range(HT):
                    ps = kp_pool.tile([128, CH], f32, tag="kp")
                    for j in range(2):
                        out = ps[:, j * 512:(j + 1) * 512]
                        # d in [0,256): one fp8 DoubleRow matmul (K=256)
                        nc.tensor.matmul(
                            out, wk8_sb[:, :, h * 128:(h + 1) * 128],
                            kt8_tiles[(b, c, j)][:],
                            start=True, stop=False,
                            perf_mode=mybir.MatmulPerfMode.DoubleRow,
                        )
                        # d in [256,512): two bf16 matmuls (K=128 each)
                        src16 = kt16_tiles[(b, c, j)]
                        for k2 in range(2):
                            nc.tensor.matmul(
                                out, wk16_sb[:, k2, h * 128:(h + 1) * 128],
                                src16[:, k2, :],
                                start=False, stop=(k2 == 1),
                            )
                    ft = feat_pool.tile([128, CH], f16, tag="ft")
                    if last:
                        # split the final tanh so the last matvec + exp can
                        # start on the first half ~0.7us earlier
                        for j in range(2):
                            nc.scalar.activation(
                                ft[:, j * 512:(j + 1) * 512],
                                ps[:, j * 512:(j + 1) * 512], Act.Tanh,
                                bias=qbias_sb[:, h, b:b + 1])
                    else:
                        nc.scalar.activation(ft[:], ps[:], Act.Tanh,
                                             bias=qbias_sb[:, h, b:b + 1])
                    fts[(b, h, c)] = ft

            def emit_matvec(c):
                # col-tiled: the 4 batches' matmuls target disjoint 32-col
                # groups of the PE array and run concurrently
                sc = sc_tiles[c]
                for h in range(HT):
                    for j in range(2):
                        for b in range(B_LOC):
                            nc.tensor.matmul(
                                sc[32 * b:32 * b + 32, j * 512:(j + 1) * 512],
                                wv_sb[:, b, h, :],
                                fts[(b, h, c)][:, j * 512:(j + 1) * 512],
                                start=(h == 0), stop=(h == HT - 1),
                                tile_position=(0, 32 * b),
                                skip_group_check=True,
                            )
                # exp/mul per 512-half: the j0 half unblocks while the j1
                # matvec is still streaming, shortening the serial tail
                for j in range(2):
                    cs = c * CH + j * 512
                    nc.scalar.activation(esc_sb[:, cs:cs + 512],
                                         sc[:, j * 512:(j + 1) * 512], Act.Exp,
                                         accum_out=psum_sb[:, 2 * c + j:2 * c + j + 1])
                    nc.vector.tensor_mul(esc_sb[:, cs:cs + 512],
                                         esc_sb[:, cs:cs + 512],
                                         vals_sb[:, cs:cs + 512])

            for c in range(NCH):
                sc_tiles[c] = sc_pool.tile([128, CH], f32, tag="sc", name="sc")
                for b in range(B_LOC):
                    emit_kproj(c, b)
                    # defer chunk c-1's matvec until after kproj(c, b0) so
                    # the last tanh's latency hides under kproj matmuls
                    if b == 0 and c > 0:
                        emit_matvec(c - 1)
            emit_matvec(NCH - 1)

            # softmax denominator; scale split DVE (3 groups) / ACT (1);
            # per-group partition-strided out DMAs overlap the later scales
            # (GpSimd tensor ops are ucode-slow ~15ns/col - never use here)
            nc.vector.reduce_sum(ssum_sb[:], psum_sb[:], axis=AX)
            nc.vector.reciprocal(recip_sb[:], ssum_sb[:])
            esc_rows = esc_sb.rearrange("(b p) n -> b p n", p=32)[:, 0, :]
            for g in [0, 1, 2, 3]:
                gs = g * CH
                if g == 1:
                    nc.scalar.mul(esc_sb[:, gs:gs + CH], esc_sb[:, gs:gs + CH],
                                  recip_sb[:])
                else:
                    nc.vector.tensor_scalar_mul(
                        esc_sb[:, gs:gs + CH], esc_sb[:, gs:gs + CH],
                        recip_sb[:])
                nc.sync.dma_start(out_ext[:, gs:gs + CH],
                                  esc_rows[:, gs:gs + CH])

    nc.compile()
    return nc


def shard_inputs(queries, keys, values, W_q, W_k, w_v):
    queries = np.asarray(queries, np.float32)
    keys = np.asarray(keys, np.float32)
    values = np.asarray(values, np.float32)
    W_q = np.asarray(W_q, np.float64)
    W_k = np.asarray(W_k, np.float32)
    w_v = np.asarray(w_v, np.float32)

    def merge_kt(w, nk, ncol):  # [nk*128, ncol] -> [128, nk*ncol] part-major
        return np.ascontiguousarray(
            w.reshape(nk, 128, ncol).transpose(1, 0, 2).reshape(128, nk * ncol))

    wk8 = merge_kt(W_k[:256], 2, H).astype(ml_dtypes.float8_e4m3fn)
    wk16 = merge_kt(W_k[256:], 2, H).astype(ml_dtypes.bfloat16)
    wv2 = np.zeros((128, B_LOC, HT, 32), np.float32)
    for b in range(B_LOC):
        for h in range(HT):
            wv2[:, b, h, 0] = w_v[h * 128:(h + 1) * 128]
    wv2 = wv2.reshape(128, B_LOC * HT * 32).astype(ml_dtypes.bfloat16)

    # qbias on host (tiny): [B, H] = queries @ W_q, exact in f64
    qb_all = (queries[:, 0, :].astype(np.float64) @ W_q).astype(np.float32)

    # [B, NK, D] -> [B, D, NK] once, then split d-ranges per core
    keysT = keys.transpose(0, 2, 1)

    in_maps = []
    for i in range(N_CORES):
        b0, b1 = i * B_LOC, (i + 1) * B_LOC
        qb = np.zeros((128, HT, B_LOC), np.float32)
        for b in range(B_LOC):
            for h in range(HT):
                qb[:, h, b] = qb_all[b0 + b, h * 128:(h + 1) * 128]
        # [B_LOC, 2, 128, NK]: slot s covers d = s*128 + p
        k8 = np.ascontiguousarray(
            keysT[b0:b1, 0:256].reshape(B_LOC, 2, 128, NK)
        ).astype(ml_dtypes.float8_e4m3fn)
        k16 = np.ascontiguousarray(
            keysT[b0:b1, 256:512].reshape(B_LOC, 2, 128, NK)
        ).astype(ml_dtypes.bfloat16)
        in_maps.append({
            "keys8": k8, "keys16": k16,
            "qbias": qb.reshape(128, HT * B_LOC),
            "vals": np.ascontiguousarray(values[b0:b1, :, 0]),
            "wk8": wk8, "wk16": wk16, "wv": wv2,
        })
    return in_maps


_NC_CACHE = {}


def run(in_maps, trace=False, tmpdir=None):
    from concourse.bass_utils import run_bass_kernel_spmd

    _install_profile_hook()
    try:
        # no artifact bucket inside the container; keep traces local
        import concourse.bass_utils as bu
        bu.upload_artifacts = lambda d: "local://" + d
    except Exception:
        pass
    if "nc" not in _NC_CACHE:
        _NC_CACHE["nc"] = build_nc()
    nc = _NC_CACHE["nc"]
    return run_bass_kernel_spmd(nc, in_maps, core_ids=list(range(N_CORES)),
                                trace=trace, tmpdir=tmpdir)


def kernel(queries, keys, values, W_q, W_k, w_v):
    in_maps = shard_inputs(queries, keys, values, W_q, W_k, w_v)
    res = run(in_maps)
    return np.concatenate([res.results[i]["out"] for i in range(N_CORES)], axis=0)
